# revision 3
# baseline (speedup 1.0000x reference)
"""Trainium2 Bass kernel for AttentionM (404us baseline -> 286us).

Sharding: 4-way DP over batch x 2-way TP over heads (8 heads/core).
Host prep is dtype/layout only: x arrives pre-transposed bf16
([128 part, 8 d-chunk, 2048 tok]); q/k/v-lin weights bf16 (value weights
host-scaled by 0.5 for the tanh gate trick); v-gate weights fp8 DoubleRow
pairs with the bias folded in as a 5th contraction pair; pad-token
constants (bk*sig(bkc) block-diag, bv*sig(bvc)|1) precomputed from biases.

Engine plan (per core, ~85% busy ACT/DVE in steady state):
  PE  bf16 q/k/v-lin projections (ones-row bias matmuls), fp8-DoubleRow
      v-gates + scores (0.5 cyc/row), bf16 ctx (token-major [q,65]
      accumulators, softmax denominator via a vt ones-column), fp8 pad
      closers. No transposes anywhere.
  ACT tanh gates (sigma(g) = 0.5*(1+tanh(g/2)); tanh/exp/copy share one
      activation table -> zero table reloads) + ~55% of the 256 softmax
      Exp tiles ([128,1024], scale=1/8, no max-subtraction needed).
  DVE gating writes (fp8 qT/kT, bf16 vt), the other exp tiles via the
      Schraudolph bit-trick (int16 = 23.083*s + 16250.49 -> bf16 exp bits,
      +-3.3% sawtooth), batched reciprocal + stride-0 broadcast-multiply
      epilogues.
  Pool/SWDGE: weight loads, xT->fp8 conversion, memsets.

Schedule: per projection block emit k then q chunks and stage per-head
packed fp8 kh/qh slices immediately, so the DoubleRow score stream (and
both exp engines) start ~10us in, interleaved with the projection tail;
v tiles are produced inside the first attention pass's slots. Scores
rotate over three [128,1024] psum tiles so consecutive exps on ACT and
DVE overlap; ctx matmuls lag 4 slots behind exp so the in-order PE
stream never waits on an exp. The [128, 8, 65] psum accumulator is
zeroed explicitly (psum start=True zeroes whole 2KB banks) and closed by
the analytic pad rank-1 update (16 identical zero-row pad tokens, ln16
folded into the pad exp bias).
"""

import sys

if "/opt/trn_rl_repo" not in sys.path:
    sys.path.insert(0, "/opt/trn_rl_repo")

import numpy as np

import concourse.bacc as bacc
import concourse.bass as cbass
import concourse.mybir as mybir
import concourse.tile as tile
from concourse.bass_utils import run_bass_kernel_spmd

F32 = mybir.dt.float32
BF16 = mybir.dt.bfloat16
FP8 = mybir.dt.float8e4
I16 = mybir.dt.int16
AF = mybir.ActivationFunctionType
ALU = mybir.AluOpType
DR = mybir.MatmulPerfMode.DoubleRow

B = 4
S = 2048
D = 1024
DC = 8
HL = 8
QL = 16
VL = 64
NKT = 16
NQT = 8            # q tiles per pass (pass q-range = 1024)
SCALE = 0.125
LN16 = float(np.log(16.0))
# Schraudolph bf16 exp bits: i16 = A16*s + B16 ; bitcast -> ~exp(s/8)
A16 = 128.0 * 1.4426950408889634 * SCALE
B16 = 16250.49

CA_EXP = 1024 * 0.8333 + 200.0     # ACT [128,1024] exp
CD_EXP = 1024 * 1.0417 + 195.0     # DVE [128,1024] Schraudolph
CA_CP = 512 * 0.8333 + 200.0       # ACT [128,512] copy/tanh
CD_CP2 = 512 * 1.0417 * 0.5 + 195.0  # DVE [128,512] bf16 2x copy
CD_CP = 512 * 1.0417 + 195.0       # DVE [128,512] 1x


def _build(repeat=1):
    nc = bacc.Bacc("TRN2", target_bir_lowering=False, debug=False, num_devices=8)

    x_d = nc.dram_tensor("xT16", [128, DC, S], BF16, kind="ExternalInput").ap()
    wq_d = nc.dram_tensor("wq16", [128, DC, 128], BF16, kind="ExternalInput").ap()
    wqc_d = nc.dram_tensor("wqc16", [128, DC, 128], BF16, kind="ExternalInput").ap()
    wk_d = nc.dram_tensor("wk16", [128, DC, 128], BF16, kind="ExternalInput").ap()
    wkc_d = nc.dram_tensor("wkc16", [128, DC, 128], BF16, kind="ExternalInput").ap()
    wv_d = nc.dram_tensor("wv16", [128, DC, 512], BF16, kind="ExternalInput").ap()
    wvc8_d = nc.dram_tensor("wvc8", [128, 5, 2, 512], FP8, kind="ExternalInput").ap()
    bq_d = nc.dram_tensor("bq16", [1, 128], BF16, kind="ExternalInput").ap()
    bk_d = nc.dram_tensor("bk16", [1, 128], BF16, kind="ExternalInput").ap()
    bv_d = nc.dram_tensor("bv16", [1, 512], BF16, kind="ExternalInput").ap()
    ones_d = nc.dram_tensor("ones16", [1, 512], BF16, kind="ExternalInput").ap()
    bqc_d = nc.dram_tensor("bqc_s", [128], F32, kind="ExternalInput").ap()
    bkc_d = nc.dram_tensor("bkc_s", [128], F32, kind="ExternalInput").ap()
    padk8_d = nc.dram_tensor("padk8", [128, HL], FP8, kind="ExternalInput").ap()
    vpad8_d = nc.dram_tensor("vpad8", [1, HL, VL + 1], FP8, kind="ExternalInput").ap()
    y_d = nc.dram_tensor("y", [S, 512], F32, kind="ExternalOutput").ap()

    with tile.TileContext(nc) as tc:
        for _ in range(repeat):
            _emit(nc, tc, x_d, wq_d, wqc_d, wk_d, wkc_d, wv_d, wvc8_d,
                  bq_d, bk_d, bv_d, ones_d, bqc_d, bkc_d, padk8_d, vpad8_d, y_d)
    nc.compile()
    return nc


def _emit(nc, tc, x_d, wq_d, wqc_d, wk_d, wkc_d, wv_d, wvc8_d,
          bq_d, bk_d, bv_d, ones_d, bqc_d, bkc_d, padk8_d, vpad8_d, y_d):
    busy = {"act": 0.0, "dve": 0.0}

    def pick(cost_a, cost_d):
        if busy["act"] + cost_a <= busy["dve"] + cost_d:
            busy["act"] += cost_a
            return "act"
        busy["dve"] += cost_d
        return "dve"

    const = tc.alloc_tile_pool(name="const", bufs=1)
    proj = tc.alloc_tile_pool(name="proj", bufs=1)
    wpool = tc.alloc_tile_pool(name="wpool", bufs=1)
    sigp = tc.alloc_tile_pool(name="sigp", bufs=2)
    # PSUM budget, bank-granular: tp 2x[128,1024]bf16 (2 banks, released
    # after proj and recycled as a second acc buffer) + sc 2x[128,1024]f32
    # (4 banks, shared by qk/scores/pad/vlin/vgate) + acc [128,8,65] (2) = 8

    # ---------------- constants / weights ----------------
    def ppart(bias_d):
        t = const.tile([128, 1], F32, name=f"b_{bias_d.name}")
        nc.sync.dma_start(out=t, in_=bias_d.unsqueeze(-1))
        return t

    bqc_sb = ppart(bqc_d)
    bkc_sb = ppart(bkc_d)
    bq_sb = const.tile([1, 128], BF16, name="bq_sb")
    nc.sync.dma_start(out=bq_sb, in_=bq_d)
    bk_sb = const.tile([1, 128], BF16, name="bk_sb")
    nc.sync.dma_start(out=bk_sb, in_=bk_d)
    bv_sb = const.tile([1, 512], BF16, name="bv_sb")
    nc.sync.dma_start(out=bv_sb, in_=bv_d)
    ones_sb = const.tile([1, 512], BF16, name="ones_sb")
    nc.sync.dma_start(out=ones_sb, in_=ones_d)
    padk8 = const.tile([128, HL], FP8, name="padk8")
    nc.sync.dma_start(out=padk8, in_=padk8_d)
    vpad8 = const.tile([1, HL, VL + 1], FP8, name="vpad8")
    nc.sync.dma_start(out=vpad8, in_=vpad8_d)

    wq = wpool.tile([128, DC, 128], BF16, name="wq")
    wqc = wpool.tile([128, DC, 128], BF16, name="wqc")
    wk = wpool.tile([128, DC, 128], BF16, name="wk")
    wkc = wpool.tile([128, DC, 128], BF16, name="wkc")
    wv = wpool.tile([128, DC, 512], BF16, name="wv")
    wvc8 = wpool.tile([128, 5, 2, 512], FP8, name="wvc8")
    nc.gpsimd.dma_start(out=wk, in_=wk_d)
    nc.gpsimd.dma_start(out=wkc, in_=wkc_d)
    nc.gpsimd.dma_start(out=wv, in_=wv_d)
    nc.gpsimd.dma_start(out=wq, in_=wq_d)
    nc.gpsimd.dma_start(out=wqc, in_=wqc_d)
    nc.gpsimd.dma_start(out=wvc8, in_=wvc8_d)

    xT8pad = const.tile([128, 2, 128], FP8, name="xT8pad")
    nc.gpsimd.memset(xT8pad, 0.0)
    nc.gpsimd.memset(xT8pad[0:1, 0, :], 1.0)

    # ---------------- projection-phase tiles ----------------
    xT = proj.tile([128, DC, S], BF16, name="xT")
    xT8 = proj.tile([128, DC, S], FP8, name="xT8")
    kT8 = proj.tile([128, S], FP8, name="kT8")
    qT8 = proj.tile([128, S], FP8, name="qT8")
    vlin = proj.tile([128, NKT, 512], BF16, name="vlin")
    # vt: [k-token-in-tile, ktile, head, 64 v + ones]
    vt = proj.tile([128, NKT, HL, VL + 1], BF16, name="vt")
    nc.gpsimd.memset(vt[:, :, :, VL:VL + 1], 1.0)

    def load_x_block(blk):
        cols = slice(blk * 512, (blk + 1) * 512)
        nc.sync.dma_start(out=xT[:, :, cols], in_=x_d[:, :, cols])

    def emit_qk_chunk(wl, wcl, brow, bcl, dest, blk):
        """dest[:, blk cols] = fp8((lin + b)/2 * (1 + tanh(gate/2)))"""
        c0 = blk * 512
        cols = slice(c0, c0 + 512)
        ps = pp_sc.tile([128, 1024], F32, name="scps")
        for d in range(DC):
            nc.tensor.matmul(ps[:, 0:512], wl[:, d, :], xT[:, d, cols],
                             start=(d == 0), stop=False)
        nc.tensor.matmul(ps[:, 0:512], brow, ones_sb, start=False, stop=True)
        for d in range(DC):
            nc.tensor.matmul(ps[:, 512:1024], wcl[:, d, :], xT[:, d, cols],
                             start=(d == 0), stop=(d == DC - 1))
        th = sigp.tile([128, 512], BF16, name="th")
        nc.scalar.activation(out=th, in_=ps[:, 512:1024], func=AF.Tanh,
                             scale=0.5, bias=bcl)
        busy["act"] += CA_CP
        nc.vector.scalar_tensor_tensor(
            out=dest[:, cols], in0=th, scalar=1.0,
            in1=ps[:, 0:512], op0=ALU.add, op1=ALU.mult)
        busy["dve"] += CD_CP

    def emit_vlin_tile(t):
        """vlin[:, t, :] = bf16((x@Wv + bv)/2), token-major."""
        ps = pp_sc.tile([128, 1024], F32, name="scps")[:, 0:512]
        toks = slice(t * 128, (t + 1) * 128)
        for d in range(DC):
            nc.tensor.matmul(ps, xT[:, d, toks], wv[:, d, :],
                             start=(d == 0), stop=False)
        nc.tensor.matmul(ps, ones_sb[:, 0:128], bv_sb, start=False, stop=True)
        dst = vlin[:, t, :]
        if pick(CA_CP, CD_CP) == "act":
            nc.scalar.activation(out=dst, in_=ps, func=AF.Copy)
        else:
            nc.vector.tensor_copy(out=dst, in_=ps)

    # ---------------- attention pools (coexist with proj tail) ----------------
    pp_sc = tc.alloc_tile_pool(name="pp_sc", bufs=2, space="PSUM")
    pp_acc = tc.alloc_tile_pool(name="pp_acc", bufs=1, space="PSUM")
    khp = tc.alloc_tile_pool(name="khp", bufs=8)
    qhp = tc.alloc_tile_pool(name="qhp", bufs=4)
    etp = tc.alloc_tile_pool(name="etp", bufs=10)
    outp = tc.alloc_tile_pool(name="outp", bufs=2)
    rcp = tc.alloc_tile_pool(name="rcp", bufs=2)
    sigv = tc.alloc_tile_pool(name="sigv", bufs=2)
    pp_sc2 = [tc.alloc_tile_pool(name="pp_sc2", bufs=1, space="PSUM")]

    kh8s = [khp.tile([8, 2, S], FP8, name="kh8") for _ in range(HL)]

    def stage_kh_block(h, blk):
        cols = slice(blk * 512, (blk + 1) * 512)
        nc.sync.dma_start(out=kh8s[h][:, :, cols],
                          in_=kT8[h * QL:(h + 1) * QL, cols])

    def stage_qh(h, qr):
        qh8 = qhp.tile([8, 2, 1024], FP8, name="qh8")
        nc.sync.dma_start(
            out=qh8, in_=qT8[h * QL:(h + 1) * QL, qr * 1024:(qr + 1) * 1024])
        return qh8

    def emit_vgate(t):
        """vt[:, t, :, 0:64] = vlin * (1 + tanh(gate/2)); gate via fp8 DR."""
        ps = pp_sc.tile([128, 1024], F32, name="scps")[:, 0:512]
        toks = slice(t * 128, (t + 1) * 128)
        for a in range(4):
            nc.tensor.matmul(ps, xT8[:, 2 * a:2 * a + 2, toks],
                             wvc8[:, a, :, :], start=(a == 0), stop=False,
                             perf_mode=DR)
        nc.tensor.matmul(ps, xT8pad, wvc8[:, 4, :, :],
                         start=False, stop=True, perf_mode=DR)
        th = sigv.tile([128, 512], BF16, name="thv")
        nc.scalar.activation(out=th, in_=ps, func=AF.Tanh, scale=0.5)
        busy["act"] += CA_CP
        nc.vector.scalar_tensor_tensor(
            out=vt[:, t, :, 0:VL],
            in0=th.rearrange("p (h v) -> p h v", h=HL), scalar=1.0,
            in1=vlin[:, t, :].rearrange("p (h v) -> p h v", h=HL),
            op0=ALU.add, op1=ALU.mult)
        busy["dve"] += CD_CP2

    sc_rr = [0]

    def emit_scores_exp(kh8, qh8, kt, force=None):
        sc_rr[0] += 1
        if pp_sc2[0] is not None and sc_rr[0] % 3 == 0:
            sc = pp_sc2[0].tile([128, 1024], F32, name="scps2")
        else:
            sc = pp_sc.tile([128, 1024], F32, name="scps")
        for j in range(2):
            nc.tensor.matmul(
                sc[:, j * 512:(j + 1) * 512],
                kh8[:, :, kt * 128:(kt + 1) * 128],
                qh8[:, :, j * 512:(j + 1) * 512],
                start=True, stop=True, perf_mode=DR)
        et = etp.tile([128, 1024], BF16, name="et")
        eng = force or pick(CA_EXP, CD_EXP)
        if force == "act":
            busy["act"] += CA_EXP
        elif force == "dve":
            busy["dve"] += CD_EXP
        if eng == "act":
            nc.scalar.activation(out=et, in_=sc, func=AF.Exp, scale=SCALE)
        else:
            nc.vector.tensor_scalar(
                out=et.bitcast(I16), in0=sc,
                scalar1=A16, scalar2=B16, op0=ALU.mult, op1=ALU.add)
        return et, eng

    def emit_zero(acc):
        # psum start=True zeroes whole 2KB banks (would clobber neighbors);
        # zero explicitly and accumulate with start=False throughout
        if pick(520 * 0.8333 + 200, 520 * 1.0417 + 195) == "act":
            au = acc.bitcast(mybir.dt.uint32)
            nc.scalar.activation(out=au, in_=au, func=AF.Copy, scale=0.0)
        else:
            nc.vector.memset(acc, 0.0)

    def emit_ctx(acc, h, et, kt):
        for qt in range(NQT):
            nc.tensor.matmul(
                acc[:, qt, :],
                et[:, qt * 128:(qt + 1) * 128],
                vt[:, kt, h, :],
                start=False, stop=False, skip_group_check=True)

    def emit_close_epilogue(h, qr, acc):
        e_padT = e_padT_ref[0]
        for qt in range(NQT):
            nc.tensor.matmul(
                acc[:, qt, :],
                e_padT[0:1, h, qr * 1024 + qt * 128: qr * 1024 + (qt + 1) * 128],
                vpad8[:, h, :],
                start=False, stop=True, skip_group_check=True)
        rc = rcp.tile([128, NQT, 1], F32, name="rc")
        nc.vector.reciprocal(out=rc, in_=acc[:, :, VL:VL + 1])
        busy["dve"] += 8 * 1.0417 + 320
        out_sb = outp.tile([128, NQT, VL], F32, name="out_sb")
        rc_b = cbass.AP(rc.tensor, rc.offset, [rc.ap[0], rc.ap[1], [0, VL]])
        nc.vector.tensor_tensor(out=out_sb, in0=acc[:, :, 0:VL], in1=rc_b,
                                op=ALU.mult)
        busy["dve"] += 512 * 1.0417 + 195
        yr = y_d[qr * 1024:(qr + 1) * 1024,
                 h * VL:(h + 1) * VL].rearrange("(a p) c -> p a c", p=128)
        nc.sync.dma_start(out=yr, in_=out_sb)

    # ---- pad path (needs full qT8; emitted after q3 below) ----
    def emit_pad():
        e_pad = proj.tile([HL, S], FP8, name="e_pad")
        ln16_sb = const.tile([128, 1], F32, name="ln16")
        nc.scalar.activation(out=ln16_sb, in_=bqc_sb, func=AF.Copy,
                             scale=0.0, bias=LN16)
        for half in range(2):
            pps = pp_sc.tile([128, 1024], F32, name="scps")
            for j in range(2):
                c0 = half * 1024 + j * 512
                nc.tensor.matmul(pps[0:HL, j * 512:(j + 1) * 512], padk8,
                                 qT8[:, c0:c0 + 512], start=True, stop=True)
            nc.scalar.activation(out=e_pad[:, half * 1024:(half + 1) * 1024],
                                 in_=pps[0:HL, :], func=AF.Exp, scale=SCALE,
                                 bias=ln16_sb[0:HL, :])
            busy["act"] += CA_EXP
        e_padT = proj.tile([1, HL, S], FP8, name="e_padT")
        nc.gpsimd.dma_start(out=e_padT, in_=e_pad)
        return e_padT

    # ---- attention slot machinery (interleaves with proj tail) ----
    # pass list: (h, qr) with qr=0 passes first (they only need q blocks 0-1)
    passes = [(h, 0) for h in range(HL)] + [(h, 1) for h in range(HL)]
    state = {"pi": 0, "cur": None, "done": 0}
    v_emitted = [0]
    e_padT_ref = [None]

    def pass_ready(pi, blocks_done):
        if pi >= len(passes):
            return False
        h, qr = passes[pi]
        kblocks = 4  # kh8s staged per block; scores kt need block kt//4
        return blocks_done >= (2 if qr == 0 else 4)

    qh_cache = {}

    def get_qh(pi):
        if pi not in qh_cache:
            h, qr = passes[pi]
            qh_cache[pi] = stage_qh(h, qr)
        return qh_cache[pi]

    def open_pass(pi):
        h, qr = passes[pi]
        acc = pp_acc.tile([128, NQT, VL + 1], F32, name="acc")
        emit_zero(acc)
        qh8 = get_qh(pi)
        # deep ctx lag: ctx(kt-4) depends on a 4-slot-old exp, so the
        # in-order PE stream never stalls waiting for the latest exp
        return {"h": h, "qr": qr, "acc": acc, "qh8": qh8, "kt": 0,
                "pend": [], "lag": 4}

    def pump_slot(blocks_done):
        """Emit one (scores+exp [+vgate][+ctx]) slot if possible."""
        st = state["cur"]
        if st is None:
            if not pass_ready(state["pi"], blocks_done):
                return False
            st = state["cur"] = open_pass(state["pi"])
        h, qr = st["h"], st["qr"]
        kt = st["kt"]
        if kt < NKT and kt // 4 < blocks_done or (kt < NKT and blocks_done == 4):
            if v_emitted[0] < NKT and blocks_done >= 1 + v_emitted[0] // 4:
                emit_vgate(v_emitted[0])
                v_emitted[0] += 1
            # pair engines: alternate forced assignment for throughput
            eng = "act" if (kt + (qr * 3) + h) % 2 == 0 else "dve"
            # rebalance drift: every 8th slot use greedy instead
            force = None if kt % 8 == 7 else eng
            et, _ = emit_scores_exp(kh8s[h], st["qh8"], kt, force=force)
            if kt == 8 and blocks_done == 4:
                get_qh(state["pi"] + 1) if state["pi"] + 1 < len(passes) else None
            st["pend"].append((et, kt))
            while len(st["pend"]) > st["lag"]:
                e, k = st["pend"].pop(0)
                emit_ctx(st["acc"], h, e, k)
            st["kt"] += 1
            return True
        if kt >= NKT:
            for e, k in st["pend"]:
                emit_ctx(st["acc"], h, e, k)
            emit_close_epilogue(h, qr, st["acc"])
            state["pi"] += 1
            state["cur"] = None
            state["done"] += 1
            return True
        return False

    # ---- projection blocks interleaved with attention slots ----
    for blk in range(4):
        load_x_block(blk)
        emit_qk_chunk(wk, wkc, bk_sb, bkc_sb, kT8, blk)
        for h in range(HL):
            stage_kh_block(h, blk)
        emit_qk_chunk(wq, wqc, bq_sb, bqc_sb, qT8, blk)
        cols = slice(blk * 512, blk * 512 + 512)
        nc.gpsimd.tensor_copy(out=xT8[:, :, cols], in_=xT[:, :, cols])
        for t in range(blk * 4, blk * 4 + 4):
            emit_vlin_tile(t)
        if blk == 3:
            e_padT_ref[0] = emit_pad()
        # interleave attention slots into the proj tail (~5 per block)
        if blk >= 1:
            for _ in range(5):
                pump_slot(blk + 1)

    # drain all remaining passes
    while state["pi"] < len(passes):
        pump_slot(4)

    pp_sc2[0].release()
    for p in (sigv, rcp, outp, etp, qhp, khp, pp_acc, pp_sc,
              sigp, wpool, proj, const):
        p.release()


_NC = None


def _get_nc():
    global _NC
    if _NC is None:
        _NC = _build()
    return _NC


def _sigmoid(v):
    return 1.0 / (1.0 + np.exp(-v))


def _shard_inputs(inputs):
    f8 = mybir.dt.np(FP8)
    b16 = mybir.dt.np(BF16)
    x = np.ascontiguousarray(np.asarray(inputs["x"], dtype=np.float32))

    def qk_w(W):
        return np.ascontiguousarray(
            W.reshape(DC, 128, 128).transpose(1, 0, 2).astype(b16))

    def v_w(W):
        return np.ascontiguousarray(
            W.reshape(DC, 128, 512).transpose(1, 0, 2).astype(b16))

    def vc_w8(W, b):
        out = np.zeros((128, 5, 2, 512), np.float32)
        out[:, 0:4, :, :] = W.reshape(4, 2, 128, 512).transpose(2, 0, 1, 3)
        out[0, 4, 0, :] = b
        return np.ascontiguousarray(out.astype(f8))

    in_maps = []
    for c in range(8):
        b, hg = c // 2, c % 2
        qk = slice(hg * 128, (hg + 1) * 128)
        vv = slice(hg * 512, (hg + 1) * 512)
        bk = np.asarray(inputs["bk"][qk], np.float32)
        bkc = np.asarray(inputs["bkc"][qk], np.float32)
        bv = np.asarray(inputs["bv"][vv], np.float32)
        bvc = np.asarray(inputs["bvc"][vv], np.float32)
        pk = (bk * _sigmoid(bkc)).astype(np.float32)
        padk8 = np.zeros((128, HL), np.float32)
        for h in range(HL):
            padk8[h * QL:(h + 1) * QL, h] = pk[h * QL:(h + 1) * QL]
        vp = (bv * _sigmoid(bvc)).reshape(HL, VL)
        vpad8 = np.ones((1, HL, VL + 1), np.float32)
        vpad8[0, :, 0:VL] = vp
        xTh = np.ascontiguousarray(
            x[b].T.reshape(DC, 128, S).transpose(1, 0, 2))
        in_maps.append({
            "xT16": xTh.astype(b16),
            "wq16": qk_w(np.asarray(inputs["Wq"][:, qk], np.float32) * 0.5),
            "wqc16": qk_w(np.asarray(inputs["Wqc"][:, qk], np.float32)),
            "wk16": qk_w(np.asarray(inputs["Wk"][:, qk], np.float32) * 0.5),
            "wkc16": qk_w(np.asarray(inputs["Wkc"][:, qk], np.float32)),
            "wv16": v_w(np.asarray(inputs["Wv"][:, vv], np.float32) * 0.5),
            "wvc8": vc_w8(np.asarray(inputs["Wvc"][:, vv], np.float32), bvc),
            "bq16": np.ascontiguousarray(
                (inputs["bq"][qk] * 0.5).astype(b16).reshape(1, 128)),
            "bk16": np.ascontiguousarray((bk * 0.5).astype(b16).reshape(1, 128)),
            "bv16": np.ascontiguousarray((bv * 0.5).astype(b16).reshape(1, 512)),
            "ones16": np.ones((1, 512), np.float32).astype(b16),
            "bqc_s": np.ascontiguousarray(
                np.asarray(inputs["bqc"][qk], np.float32) * 0.5),
            "bkc_s": np.ascontiguousarray(bkc * 0.5),
            "padk8": padk8.astype(f8),
            "vpad8": vpad8.astype(f8),
        })
    return in_maps


def kernel(**inputs) -> np.ndarray:
    nc = _get_nc()
    in_maps = _shard_inputs(inputs)
    res = run_bass_kernel_spmd(nc, in_maps, list(range(8)))
    out = np.empty((B, S, 1024), np.float32)
    for c in range(8):
        b, hg = c // 2, c % 2
        out[b, :, hg * 512:(hg + 1) * 512] = res.results[c]["y"]
    return out


if __name__ == "__main__":
    rng = np.random.default_rng(0)
    d = 1.0 / np.sqrt(D)
    inputs = {
        "x": rng.standard_normal((B, S, D), dtype=np.float32),
        "Wq": rng.standard_normal((D, 256), dtype=np.float32) * d,
        "bq": rng.standard_normal(256).astype(np.float32) * 0.02,
        "Wqc": rng.standard_normal((D, 256), dtype=np.float32) * d,
        "bqc": rng.standard_normal(256).astype(np.float32) * 0.02,
        "Wk": rng.standard_normal((D, 256), dtype=np.float32) * d,
        "bk": rng.standard_normal(256).astype(np.float32) * 0.02,
        "Wkc": rng.standard_normal((D, 256), dtype=np.float32) * d,
        "bkc": rng.standard_normal(256).astype(np.float32) * 0.02,
        "Wv": rng.standard_normal((D, 1024), dtype=np.float32) * d,
        "bv": rng.standard_normal(1024).astype(np.float32) * 0.02,
        "Wvc": rng.standard_normal((D, 1024), dtype=np.float32) * d,
        "bvc": rng.standard_normal(1024).astype(np.float32) * 0.02,
    }
    y = kernel(**inputs)
    print("kernel output", y.shape, y.dtype, float(np.abs(y).max()))
